# revision 30
# baseline (speedup 1.0000x reference)
"""Multi-head dilated sliding-window attention (window=129, dil=1) on 8 TRN2 cores.

Sharding: sequence-parallel. Each core computes 256 query rows (N=2048 / 8),
with a 64-row K/V halo on each side (zero-padded at the sequence edges).
Weights are replicated (resident in SBUF, bf16).

Band-softmax identity used (reference softmaxes the FULL row with zeros
outside the band):
    out_i = (sum_band (e^{s_ij} - 1) V_j + sum_all V_j) / (sum_band (e^{s_ij} - 1) + N)
with V_raw = x@Wv (no bias; bv is folded into the output bias host-side:
bo' = bv@Wo + bo), bk applied only to real (non-padding) K rows via an
indicator-row matmul, and the global sum_all V_j = (sum_n x_n) @ Wv computed
HOST-side (tiny matvec) and shipped as part of the per-head bias row.

Compute dtype: bf16 operands into the PE, fp32 PSUM accumulation; the
post-exp chain runs in bf16 for 2x DVE rate.

Structure: per head-pair round db: Q^T/K^T projections, the previous
round's PV flush + normalize, this round's scores + exp/-1/mask chain,
and the previous round's A-transpose (feeding the output projection) all
interleave so the PE stays dense and the HAM clock never down-gates.
Input DMAs are dispatched from BOTH hardware-DGE engines (sync + scalar)
to beat the ~0.6us-per-dispatch serialization.
"""

import numpy as np
import ml_dtypes
from contextlib import ExitStack

import concourse.bass as bass
import concourse.tile as tile
from concourse import bacc, mybir
from concourse.bass_utils import run_bass_kernel_spmd

F32 = mybir.dt.float32
BF16 = mybir.dt.bfloat16
NPBF16 = ml_dtypes.bfloat16
N, E, H, D = 2048, 1024, 16, 64
R = N // 8          # 256 query rows per core
HALO = R + 128      # 384 K/V rows per core
NQB = R // 128      # query blocks per core


def build_graph():
    nc = bacc.Bacc("TRN2", target_bir_lowering=False, debug=False, num_devices=8)

    xh_d = nc.declare_dram_parameter("xh", [HALO, E], BF16, isOutput=False)
    xvalid_d = nc.declare_dram_parameter("xvalid", [1, HALO], BF16, isOutput=False)
    wq_d = nc.declare_dram_parameter("Wq", [E, H * D], BF16, isOutput=False)
    wk_d = nc.declare_dram_parameter("Wk", [E, H * D], BF16, isOutput=False)
    wv_d = nc.declare_dram_parameter("Wv", [E, H * D], BF16, isOutput=False)
    wo_d = nc.declare_dram_parameter("Wo", [H * D, E], BF16, isOutput=False)
    bq_d = nc.declare_dram_parameter("bq_r", [128, 8], F32, isOutput=False)
    bk_d = nc.declare_dram_parameter("bk_row", [1, H * D], BF16, isOutput=False)
    bo_d = nc.declare_dram_parameter("bo_row", [1, E], BF16, isOutput=False)
    bc_d = nc.declare_dram_parameter("biascat_row", [1, H * (D + 1)], BF16,
                                     isOutput=False)
    m4_d = nc.declare_dram_parameter("mask4", [128, 512], BF16, isOutput=False)
    id_d = nc.declare_dram_parameter("ident", [128, 128], BF16, isOutput=False)
    out_d = nc.declare_dram_parameter("out", [R, E], BF16, isOutput=True)

    with tile.TileContext(nc) as tc, ExitStack() as ctx:
        const = ctx.enter_context(tc.tile_pool(name="const", bufs=1))
        pers = ctx.enter_context(tc.tile_pool(name="pers", bufs=1))
        epool = ctx.enter_context(tc.tile_pool(name="epool", bufs=3))
        ppool = ctx.enter_context(tc.tile_pool(name="ppool", bufs=5))
        zpool = ctx.enter_context(tc.tile_pool(name="zpool", bufs=4))
        obpool = ctx.enter_context(tc.tile_pool(name="obpool", bufs=2))
        psum = ctx.enter_context(tc.tile_pool(name="psum", bufs=8, space="PSUM"))

        def ps(shape, dt=F32):
            return psum.tile(shape, dt, tag="ps", name="pst")

        # ---- tiles --------------------------------------------------------
        xtiles = [const.tile([128, E], BF16, tag=f"xload{st}", name="xt")
                  for st in range(3)]
        identity = const.tile([128, 128], BF16, tag="identity")
        wv_t = [const.tile([128, E], BF16, tag=f"wv{et}", name="wt")
                for et in range(8)]
        wq_t = [const.tile([128, E], BF16, tag=f"wq{et}", name="wt")
                for et in range(8)]
        wk_t = [const.tile([128, E], BF16, tag=f"wk{et}", name="wt")
                for et in range(8)]
        wo_t = [const.tile([128, E], BF16, tag=f"wo{et}", name="wt")
                for et in range(8)]
        m4 = const.tile([128, 512], BF16, tag="m4")
        bq_sb = const.tile([128, 8], F32, tag="bq")
        bk_sb = const.tile([1, H * D], BF16, tag="bk")
        bo_sb = const.tile([1, E], BF16, tag="bo")
        biascat = const.tile([1, H, D + 1], BF16, tag="biascat")
        valid_sb = const.tile([1, HALO], BF16, tag="valid")

        # ---- input DMA dispatch, two hardware-DGE engines in parallel -----
        # sync: x (earliest need), small consts, then wq/wk pairs in round
        # order. scalar: ident + wv (V-phase need), later wo.
        nc.sync.dma_start(identity[:], id_d[:, :])
        nc.scalar.dma_start(wv_t[0][:], wv_d[0:128, :])
        for st in range(3):
            nc.sync.dma_start(xtiles[st][:, 0:512],
                              xh_d[st * 128:(st + 1) * 128, 0:512])
            nc.scalar.dma_start(xtiles[st][:, 512:1024],
                                xh_d[st * 128:(st + 1) * 128, 512:1024])
        for et in range(1, 8):
            nc.scalar.dma_start(wv_t[et][:], wv_d[et * 128:(et + 1) * 128, :])
        for et in range(2):
            nc.sync.dma_start(wq_t[et][:], wq_d[et * 128:(et + 1) * 128, :])
            nc.sync.dma_start(wk_t[et][:], wk_d[et * 128:(et + 1) * 128, :])
        nc.sync.dma_start(bq_sb[:], bq_d[:, :])
        nc.sync.dma_start(bk_sb[:], bk_d[:, :])
        nc.sync.dma_start(valid_sb[:], xvalid_d[:, :])
        nc.sync.dma_start(m4[:], m4_d[:, :])
        nc.sync.dma_start(wq_t[2][:], wq_d[256:384, :])
        nc.sync.dma_start(wk_t[2][:], wk_d[256:384, :])
        nc.sync.dma_start(biascat[:].rearrange("a h d -> a (h d)"), bc_d[:, :])
        nc.sync.dma_start(bo_sb[:], bo_d[:, :])
        for et in range(3, 8):
            nc.sync.dma_start(wq_t[et][:], wq_d[et * 128:(et + 1) * 128, :])
            nc.sync.dma_start(wk_t[et][:], wk_d[et * 128:(et + 1) * 128, :])

        # ---- PE clock warm-up while the x DMAs land -----------------------
        wu = const.tile([128, 128], BF16, tag="wu")
        nc.vector.memset(wu[:], 0.0)
        wups = psum.tile([128, 128], F32, tag="ps", name="wups")
        for _ in range(12):
            nc.tensor.matmul(wups[:], wu[:], wu[:], start=True, stop=True)

        ones_sb = const.tile([1, 128], BF16, tag="ones")
        nc.vector.memset(ones_sb[:], 1.0)

        # ---- persistent activations ---------------------------------------
        xT = pers.tile([128, 8, HALO], BF16, tag="xT")       # [e_p, e_t, seq]
        QT = pers.tile([128, 8, R], BF16, tag="QT")          # [d_p, d_t, q]
        KT = pers.tile([128, 8, HALO], BF16, tag="KT")       # [d_p, d_t, seq]
        Vaug = pers.tile([128, 3, H, D + 1], BF16, tag="Vaug")
        Asc = pers.tile([128, NQB, H * D], BF16, tag="Asc")  # [q_p, qblk, dims]
        AT = pers.tile([128, 8, R], BF16, tag="AT")          # [d_p, d_t, q]

        # ---- transpose x to xT (PE transpose) -----------------------------
        for st in range(3):
            for et in range(8):
                tp = ps([128, 128], BF16)
                nc.tensor.transpose(tp[:], xtiles[st][:, et * 128:(et + 1) * 128],
                                    identity[:])
                nc.vector.tensor_copy(xT[:, et, st * 128:(st + 1) * 128], tp[:])

        def proj(db):
            qp = ps([128, R])
            for et in range(8):
                nc.tensor.matmul(qp[:], wq_t[et][:, db * 128:(db + 1) * 128],
                                 xT[:, et, 64:64 + R],
                                 start=(et == 0), stop=(et == 7))
            nc.vector.tensor_scalar_add(QT[:, db, :], qp[:], bq_sb[:, db:db + 1])
            kp = ps([128, HALO])
            for et in range(8):
                nc.tensor.matmul(kp[:], wk_t[et][:, db * 128:(db + 1) * 128],
                                 xT[:, et, :], start=(et == 0), stop=False)
            nc.tensor.matmul(kp[:], bk_sb[0:1, db * 128:(db + 1) * 128],
                             valid_sb[0:1, :], start=False, stop=True)
            nc.scalar.copy(KT[:, db, :], kp[:])

        def sblock(db):
            # scores for both heads of pair db (two concurrent 64-row
            # groups) + the exp/-1/mask chain producing the P tiles
            ptl = {}
            for i, h in enumerate((2 * db, 2 * db + 1)):
                rr = i * 64
                sp = ps([128, 512])
                for quad in range(4):
                    qblk, cblk = quad // 2, quad % 2
                    nc.tensor.matmul(
                        sp[:, quad * 128:(quad + 1) * 128],
                        KT[rr:rr + 64, db,
                           (qblk + cblk) * 128:(qblk + cblk + 1) * 128],
                        QT[rr:rr + 64, db, qblk * 128:(qblk + 1) * 128],
                        start=(quad == 0), stop=(quad == 3))
                et_ = epool.tile([128, 512], BF16, tag="e", name="et_")
                nc.scalar.activation(et_[:], sp[:],
                                     mybir.ActivationFunctionType.Exp)
                nc.vector.tensor_scalar_add(et_[:], et_[:], -1.0)
                pt = ppool.tile([128, 512], BF16, tag="p", name="pt")
                nc.vector.tensor_mul(pt[:], et_[:], m4[:])
                ptl[h] = pt
            return ptl

        # ---- V (natural layout, raw): et-major over st 0/1 so each wv
        # chunk arrival feeds two blocks of work at once (no DMA-pacing
        # stalls); rounds 0/1's projections and round 0's scores interleave
        # to warm the pipeline; st=2 re-streams the resident wv afterwards.
        vp = [[ps([128, 512]) for _ in range(2)] for _ in range(2)]
        for et in range(8):
            for st in range(2):
                for hf in range(2):
                    nc.tensor.matmul(vp[st][hf][:],
                                     xT[:, et, st * 128:(st + 1) * 128],
                                     wv_t[et][:, hf * 512:(hf + 1) * 512],
                                     start=(et == 0), stop=(et == 7))
            if et == 1:
                proj(0)
            elif et == 3:
                proj(1)
            elif et == 5:
                ptl0 = sblock(0)
        for st in range(2):
            for hf in range(2):
                src = vp[st][hf][:].rearrange("p (h d) -> p h d", d=D)
                nc.scalar.copy(Vaug[:, st, hf * 8:(hf + 1) * 8, 0:D], src)
        vp2 = [ps([128, 512]) for _ in range(2)]
        for et in range(8):
            for hf in range(2):
                nc.tensor.matmul(vp2[hf][:],
                                 xT[:, et, 256:384],
                                 wv_t[et][:, hf * 512:(hf + 1) * 512],
                                 start=(et == 0), stop=(et == 7))
        for hf in range(2):
            src = vp2[hf][:].rearrange("p (h d) -> p h d", d=D)
            nc.scalar.copy(Vaug[:, 2, hf * 8:(hf + 1) * 8, 0:D], src)
        nc.vector.memset(Vaug[:, :, :, D:D + 1], 1.0)

        # wo dispatches ride sync's in-order tail: the DMA-sem slot
        # throttling naturally sequences them after the wq/wk transfers,
        # keeping early HBM bandwidth for the critical-path loads.
        for et in range(8):
            nc.sync.dma_start(wo_t[et][:], wo_d[et * 128:(et + 1) * 128, :])

        # ---- fused projections + banded attention, one head-pair at a time
        # round r: (1) Q^T/K^T projection for db=r, (2) PV flush + normalize
        # of round r-1, (3) scores + exp/-1/mask chain for r, (4) A-transpose
        # of round r-1 feeding the output projection.
        # Per-head p layout: [q0c0 | q0c1 | q1c0 | q1c1], quadrant j uses
        # keys halo block (qblk+cblk) and mask m0/m1 alternating.
        prev = None  # (db, ptiles{h: pt})

        def pv_flush(pr):
            # merged psum tile: [q0h0 | q0h1 | q1h0 | q1h1], 65 cols each.
            # All full-K accumulate matmuls grouped first, then the K=1 bias
            # matmuls: interleaving row-group-mode switches flushes the PE.
            db, ptl = pr
            pv = ps([128, 4 * (D + 1)])
            for qblk in range(NQB):
                for i, h in enumerate((2 * db, 2 * db + 1)):
                    off = (qblk * 2 + i) * (D + 1)
                    for cblk in range(2):
                        quad = qblk * 2 + cblk
                        nc.tensor.matmul(pv[:, off:off + D + 1],
                                         ptl[h][:, quad * 128:(quad + 1) * 128],
                                         Vaug[:, qblk + cblk, h, :],
                                         start=(qblk == 0 and i == 0
                                                and cblk == 0), stop=False)
            for qblk in range(NQB):
                for i, h in enumerate((2 * db, 2 * db + 1)):
                    off = (qblk * 2 + i) * (D + 1)
                    nc.tensor.matmul(pv[:, off:off + D + 1], ones_sb[0:1, :],
                                     biascat[0:1, h, :], start=False,
                                     stop=(qblk == 1 and i == 1))
            zinv = zpool.tile([128, 4], F32, tag="z", name="zinv")
            zsrc = pv[:].rearrange("p (a z) -> p a z", z=D + 1)[:, :, D]
            nc.vector.reciprocal(zinv[:], zsrc)
            for qblk in range(NQB):
                for i, h in enumerate((2 * db, 2 * db + 1)):
                    j = qblk * 2 + i
                    off = j * (D + 1)
                    if i == 0:
                        nc.scalar.activation(Asc[:, qblk, h * D:(h + 1) * D],
                                             pv[:, off:off + D],
                                             mybir.ActivationFunctionType.Copy,
                                             scale=zinv[:, j:j + 1])
                    else:
                        nc.vector.tensor_scalar_mul(
                            Asc[:, qblk, h * D:(h + 1) * D],
                            pv[:, off:off + D], zinv[:, j:j + 1])

        def a_transpose(db):
            tp = ps([128, 256], BF16)
            for qblk in range(NQB):
                nc.tensor.transpose(tp[:, qblk * 128:(qblk + 1) * 128],
                                    Asc[:, qblk, db * 128:(db + 1) * 128],
                                    identity[:])
            nc.vector.tensor_copy(AT[:, db, :], tp[:])

        prev = (0, ptl0)
        for r in range(1, 8 + 1):
            if r < 8:
                db = r
                if r + 1 < 8:  # proj(0)/proj(1) were done in the V phase
                    proj(r + 1)
                pv_flush(prev)
                ptl = sblock(db)
                # lag-2 A-transpose: Asc(r-2) was normalized a full round
                # ago, so the transpose never waits on the recip/scale chain
                if r >= 2:
                    a_transpose(r - 2)
                prev = (db, ptl)
            else:
                # epilogue: fill the PE while round 7's softmax chain and
                # pv/normalize complete.  qblk-major so qblk0's copy-out +
                # DMA overlap qblk1's projection matmuls.
                a_transpose(6)
                opt = {}

                def oproj(qblk, at, start):
                    opp = opt[qblk]
                    for hf in range(2):
                        nc.tensor.matmul(
                            opp[hf][:],
                            AT[:, at, qblk * 128:(qblk + 1) * 128],
                            wo_t[at][:, hf * 512:(hf + 1) * 512],
                            start=start, stop=False)

                def ofinish(qblk):
                    # per-hf pipeline: bias-stop, cast, then two quarter
                    # DMAs per half so four transfers fly concurrently
                    opp = opt[qblk]
                    r0 = qblk * 128
                    ob = obpool.tile([128, E], BF16, tag="ob")
                    nc.tensor.matmul(opp[0][:], ones_sb[0:1, :],
                                     bo_sb[0:1, 0:512], start=False, stop=True)
                    nc.vector.tensor_copy(ob[:, 0:512], opp[0][:])
                    nc.sync.dma_start(out_d[r0:r0 + 128, 0:256], ob[:, 0:256])
                    nc.scalar.dma_start(out_d[r0:r0 + 128, 256:512],
                                        ob[:, 256:512])
                    nc.tensor.matmul(opp[1][:], ones_sb[0:1, :],
                                     bo_sb[0:1, 512:1024], start=False,
                                     stop=True)
                    nc.scalar.copy(ob[:, 512:1024], opp[1][:])
                    nc.sync.dma_start(out_d[r0:r0 + 128, 512:768],
                                      ob[:, 512:768])
                    nc.scalar.dma_start(out_d[r0:r0 + 128, 768:1024],
                                        ob[:, 768:1024])

                opt[0] = [ps([128, 512]) for _ in range(2)]
                for at in range(6):
                    oproj(0, at, at == 0)
                pv_flush(prev)
                oproj(0, 6, False)
                a_transpose(7)
                oproj(0, 7, False)
                ofinish(0)
                opt[1] = [ps([128, 512]) for _ in range(2)]
                for at in range(8):
                    oproj(1, at, at == 0)
                ofinish(1)

    nc.compile()
    return nc


_NC = None


def get_nc():
    global _NC
    if _NC is None:
        _NC = build_graph()
    return _NC


def make_in_maps(x, Wq, bq, Wk, bk, Wv, bv, Wo, bo):
    f = lambda a: np.ascontiguousarray(np.asarray(a, dtype=np.float32))
    bf = lambda a: np.ascontiguousarray(
        np.asarray(a, dtype=np.float32).astype(NPBF16))
    x2 = f(x).reshape(N, E)
    ci = np.arange(128, dtype=np.float32)[:, None]  # key index c (partitions)
    qi = np.arange(128, dtype=np.float32)[None, :]  # query index q (free)
    m0 = (ci >= qi).astype(np.float32)
    m1 = (ci <= qi).astype(np.float32)
    mask4 = np.concatenate([m0, m1, m0, m1], axis=1)
    # host folds: sum_all V_j = xsum @ Wv (per-head bias row, with the +N
    # denominator count), and bo' = bv @ Wo + bo.
    sv = (x2.sum(0, dtype=np.float32) @ f(Wv)).reshape(H, D)
    biascat = np.concatenate(
        [sv, np.full((H, 1), float(N), np.float32)], axis=1).reshape(1, -1)
    bo2 = f(bv) @ f(Wo) + f(bo)
    common = {
        "Wq": bf(Wq), "Wk": bf(Wk), "Wv": bf(Wv), "Wo": bf(Wo),
        "bq_r": f(bq).reshape(8, 128).T.copy(),
        "bk_row": bf(bk).reshape(1, H * D),
        "bo_row": bf(bo2).reshape(1, E),
        "biascat_row": bf(biascat),
        "mask4": bf(mask4),
        "ident": np.eye(128, dtype=np.float32).astype(NPBF16),
    }
    in_maps = []
    for c in range(8):
        r0 = c * R
        xh = np.zeros((HALO, E), NPBF16)
        valid = np.zeros((1, HALO), NPBF16)
        lo, hi = r0 - 64, r0 + R + 64
        slo, shi = max(lo, 0), min(hi, N)
        xh[slo - lo: shi - lo] = x2[slo:shi].astype(NPBF16)
        valid[0, slo - lo: shi - lo] = 1.0
        in_maps.append({**common, "xh": xh, "xvalid": valid})
    return in_maps


def kernel(x, Wq, bq, Wk, bk, Wv, bv, Wo, bo, _trace=False, _trace_kwargs=None):
    nc = get_nc()
    in_maps = make_in_maps(x, Wq, bq, Wk, bk, Wv, bv, Wo, bo)
    res = run_bass_kernel_spmd(nc, in_maps, list(range(8)), trace=_trace,
                               **(_trace_kwargs or {}))
    out = np.concatenate([np.asarray(res.results[c]["out"]) for c in range(8)],
                         axis=0)
    kernel.last_result = res
    return out[None].astype(np.float32)


# revision 32
# speedup vs baseline: 1.0802x; 1.0802x over previous
"""Multi-head dilated sliding-window attention (window=129, dil=1) on 8 TRN2 cores.

Sharding: sequence-parallel. Each core computes 256 query rows (N=2048 / 8),
with a 64-row K/V halo on each side (zero-padded at the sequence edges).
Weights are replicated (resident in SBUF, bf16).

Band-softmax identity used (reference softmaxes the FULL row with zeros
outside the band):
    out_i = (sum_band (e^{s_ij} - 1) V_j + sum_all V_j) / (sum_band (e^{s_ij} - 1) + N)
with V_raw = x@Wv (no bias; bv is folded into the output bias host-side:
bo' = bv@Wo + bo), bk applied only to real (non-padding) K rows via an
indicator-row matmul, and the global sum_all V_j = (sum_n x_n) @ Wv computed
HOST-side (tiny matvec) and shipped as part of the per-head bias row.

Compute dtype: bf16 operands into the PE, fp32 PSUM accumulation; the
post-exp chain runs in bf16 for 2x DVE rate.

Structure: per head-pair round db: Q^T/K^T projections, the previous
round's PV flush + normalize, this round's scores + exp/-1/mask chain,
and the previous round's A-transpose (feeding the output projection) all
interleave so the PE stays dense and the HAM clock never down-gates.
Input DMAs are dispatched from BOTH hardware-DGE engines (sync + scalar)
to beat the ~0.6us-per-dispatch serialization.
"""

import numpy as np
import ml_dtypes
from contextlib import ExitStack

import concourse.bass as bass
import concourse.tile as tile
from concourse import bacc, mybir
from concourse.bass_utils import run_bass_kernel_spmd

F32 = mybir.dt.float32
BF16 = mybir.dt.bfloat16
NPBF16 = ml_dtypes.bfloat16
N, E, H, D = 2048, 1024, 16, 64
R = N // 8          # 256 query rows per core
HALO = R + 128      # 384 K/V rows per core
NQB = R // 128      # query blocks per core


def build_graph():
    nc = bacc.Bacc("TRN2", target_bir_lowering=False, debug=False, num_devices=8)

    xh_d = nc.declare_dram_parameter("xh", [HALO, E], BF16, isOutput=False)
    xvalid_d = nc.declare_dram_parameter("xvalid", [1, HALO], BF16, isOutput=False)
    wq_d = nc.declare_dram_parameter("Wq", [E, H * D], BF16, isOutput=False)
    wk_d = nc.declare_dram_parameter("Wk", [E, H * D], BF16, isOutput=False)
    wv_d = nc.declare_dram_parameter("Wv", [E, H * D], BF16, isOutput=False)
    wo_d = nc.declare_dram_parameter("Wo", [H * D, E], BF16, isOutput=False)
    bq_d = nc.declare_dram_parameter("bq_r", [128, 8], F32, isOutput=False)
    bk_d = nc.declare_dram_parameter("bk_row", [1, H * D], BF16, isOutput=False)
    bo_d = nc.declare_dram_parameter("bo_row", [1, E], BF16, isOutput=False)
    bc_d = nc.declare_dram_parameter("biascat_row", [1, H * (D + 1)], BF16,
                                     isOutput=False)
    m4_d = nc.declare_dram_parameter("mask4", [128, 512], BF16, isOutput=False)
    id_d = nc.declare_dram_parameter("ident", [128, 128], BF16, isOutput=False)
    out_d = nc.declare_dram_parameter("out", [R, E], BF16, isOutput=True)

    with tile.TileContext(nc) as tc, ExitStack() as ctx:
        const = ctx.enter_context(tc.tile_pool(name="const", bufs=1))
        pers = ctx.enter_context(tc.tile_pool(name="pers", bufs=1))
        epool = ctx.enter_context(tc.tile_pool(name="epool", bufs=3))
        ppool = ctx.enter_context(tc.tile_pool(name="ppool", bufs=5))
        zpool = ctx.enter_context(tc.tile_pool(name="zpool", bufs=4))
        obpool = ctx.enter_context(tc.tile_pool(name="obpool", bufs=2))
        psum = ctx.enter_context(tc.tile_pool(name="psum", bufs=8, space="PSUM"))

        def ps(shape, dt=F32):
            return psum.tile(shape, dt, tag="ps", name="pst")

        # ---- tiles --------------------------------------------------------
        xtiles = [const.tile([128, E], BF16, tag=f"xload{st}", name="xt")
                  for st in range(3)]
        identity = const.tile([128, 128], BF16, tag="identity")
        wv_t = [const.tile([128, E], BF16, tag=f"wv{et}", name="wt")
                for et in range(8)]
        wq_t = [const.tile([128, E], BF16, tag=f"wq{et}", name="wt")
                for et in range(8)]
        wk_t = [const.tile([128, E], BF16, tag=f"wk{et}", name="wt")
                for et in range(8)]
        wo_t = [const.tile([128, E], BF16, tag=f"wo{et}", name="wt")
                for et in range(8)]
        m4 = const.tile([128, 512], BF16, tag="m4")
        bq_sb = const.tile([128, 8], F32, tag="bq")
        bk_sb = const.tile([1, H * D], BF16, tag="bk")
        bo_sb = const.tile([1, E], BF16, tag="bo")
        biascat = const.tile([1, H, D + 1], BF16, tag="biascat")
        valid_sb = const.tile([1, HALO], BF16, tag="valid")

        # ---- input DMA dispatch, two hardware-DGE engines in parallel -----
        # sync: x (earliest need), small consts, then wq/wk pairs in round
        # order. scalar: ident + wv (V-phase need), later wo.
        nc.sync.dma_start(identity[:], id_d[:, :])
        nc.scalar.dma_start(wv_t[0][:], wv_d[0:128, :])
        for st in range(3):
            nc.sync.dma_start(xtiles[st][:, 0:512],
                              xh_d[st * 128:(st + 1) * 128, 0:512])
            nc.scalar.dma_start(xtiles[st][:, 512:1024],
                                xh_d[st * 128:(st + 1) * 128, 512:1024])
        for et in range(1, 8):
            nc.scalar.dma_start(wv_t[et][:], wv_d[et * 128:(et + 1) * 128, :])
        for et in range(2):
            nc.sync.dma_start(wq_t[et][:], wq_d[et * 128:(et + 1) * 128, :])
            nc.sync.dma_start(wk_t[et][:], wk_d[et * 128:(et + 1) * 128, :])
        nc.sync.dma_start(bq_sb[:], bq_d[:, :])
        nc.sync.dma_start(bk_sb[:], bk_d[:, :])
        nc.sync.dma_start(valid_sb[:], xvalid_d[:, :])
        nc.sync.dma_start(m4[:], m4_d[:, :])
        nc.sync.dma_start(wq_t[2][:], wq_d[256:384, :])
        nc.sync.dma_start(wk_t[2][:], wk_d[256:384, :])
        nc.sync.dma_start(biascat[:].rearrange("a h d -> a (h d)"), bc_d[:, :])
        nc.sync.dma_start(bo_sb[:], bo_d[:, :])
        for et in range(3, 8):
            nc.sync.dma_start(wq_t[et][:], wq_d[et * 128:(et + 1) * 128, :])
            nc.sync.dma_start(wk_t[et][:], wk_d[et * 128:(et + 1) * 128, :])

        # ---- PE clock warm-up while the x DMAs land -----------------------
        wu = const.tile([128, 128], BF16, tag="wu")
        nc.vector.memset(wu[:], 0.0)
        wups = psum.tile([128, 128], F32, tag="ps", name="wups")
        for _ in range(12):
            nc.tensor.matmul(wups[:], wu[:], wu[:], start=True, stop=True)

        ones_sb = const.tile([1, 128], BF16, tag="ones")
        nc.vector.memset(ones_sb[:], 1.0)

        # ---- persistent activations ---------------------------------------
        xT = pers.tile([128, 8, HALO], BF16, tag="xT")       # [e_p, e_t, seq]
        QT = pers.tile([128, 8, R], BF16, tag="QT")          # [d_p, d_t, q]
        KT = pers.tile([128, 8, HALO], BF16, tag="KT")       # [d_p, d_t, seq]
        Vaug = pers.tile([128, 3, H, D + 1], BF16, tag="Vaug")
        Asc = pers.tile([128, NQB, H * D], BF16, tag="Asc")  # [q_p, qblk, dims]
        AT = pers.tile([128, 8, R], BF16, tag="AT")          # [d_p, d_t, q]

        # ---- transpose x to xT (PE transpose) -----------------------------
        for st in range(3):
            for et in range(8):
                tp = ps([128, 128], BF16)
                nc.tensor.transpose(tp[:], xtiles[st][:, et * 128:(et + 1) * 128],
                                    identity[:])
                nc.vector.tensor_copy(xT[:, et, st * 128:(st + 1) * 128], tp[:])

        def proj(db):
            qp = ps([128, R])
            for et in range(8):
                nc.tensor.matmul(qp[:], wq_t[et][:, db * 128:(db + 1) * 128],
                                 xT[:, et, 64:64 + R],
                                 start=(et == 0), stop=(et == 7))
            nc.vector.tensor_scalar_add(QT[:, db, :], qp[:], bq_sb[:, db:db + 1])
            kp = ps([128, HALO])
            for et in range(8):
                nc.tensor.matmul(kp[:], wk_t[et][:, db * 128:(db + 1) * 128],
                                 xT[:, et, :], start=(et == 0), stop=False)
            nc.tensor.matmul(kp[:], bk_sb[0:1, db * 128:(db + 1) * 128],
                             valid_sb[0:1, :], start=False, stop=True)
            nc.scalar.copy(KT[:, db, :], kp[:])

        def sblock(db):
            # scores for both heads of pair db (two concurrent 64-row
            # groups) + the exp/-1/mask chain producing the P tiles
            ptl = {}
            for i, h in enumerate((2 * db, 2 * db + 1)):
                rr = i * 64
                sp = ps([128, 512])
                for quad in range(4):
                    qblk, cblk = quad // 2, quad % 2
                    nc.tensor.matmul(
                        sp[:, quad * 128:(quad + 1) * 128],
                        KT[rr:rr + 64, db,
                           (qblk + cblk) * 128:(qblk + cblk + 1) * 128],
                        QT[rr:rr + 64, db, qblk * 128:(qblk + 1) * 128],
                        start=(quad == 0), stop=(quad == 3))
                et_ = epool.tile([128, 512], BF16, tag="e", name="et_")
                nc.scalar.activation(et_[:], sp[:],
                                     mybir.ActivationFunctionType.Exp)
                nc.vector.tensor_scalar_add(et_[:], et_[:], -1.0)
                pt = ppool.tile([128, 512], BF16, tag="p", name="pt")
                nc.vector.tensor_mul(pt[:], et_[:], m4[:])
                ptl[h] = pt
            return ptl

        # ---- V (natural layout, raw): et-major over st 0/1 so each wv
        # chunk arrival feeds two blocks of work at once (no DMA-pacing
        # stalls); rounds 0/1's projections and round 0's scores interleave
        # to warm the pipeline; st=2 re-streams the resident wv afterwards.
        vp = [[ps([128, 512]) for _ in range(2)] for _ in range(2)]
        for et in range(8):
            for st in range(2):
                for hf in range(2):
                    nc.tensor.matmul(vp[st][hf][:],
                                     xT[:, et, st * 128:(st + 1) * 128],
                                     wv_t[et][:, hf * 512:(hf + 1) * 512],
                                     start=(et == 0), stop=(et == 7))
            if et == 1:
                proj(0)
        for st in range(2):
            for hf in range(2):
                src = vp[st][hf][:].rearrange("p (h d) -> p h d", d=D)
                nc.scalar.copy(Vaug[:, st, hf * 8:(hf + 1) * 8, 0:D], src)
        vp2 = [ps([128, 512]) for _ in range(2)]
        for et in range(8):
            for hf in range(2):
                nc.tensor.matmul(vp2[hf][:],
                                 xT[:, et, 256:384],
                                 wv_t[et][:, hf * 512:(hf + 1) * 512],
                                 start=(et == 0), stop=(et == 7))
        for hf in range(2):
            src = vp2[hf][:].rearrange("p (h d) -> p h d", d=D)
            nc.scalar.copy(Vaug[:, 2, hf * 8:(hf + 1) * 8, 0:D], src)
        nc.vector.memset(Vaug[:, :, :, D:D + 1], 1.0)

        # wo dispatches ride sync's in-order tail: the DMA-sem slot
        # throttling naturally sequences them after the wq/wk transfers,
        # keeping early HBM bandwidth for the critical-path loads.
        for et in range(8):
            nc.sync.dma_start(wo_t[et][:], wo_d[et * 128:(et + 1) * 128, :])

        # ---- fused projections + banded attention, one head-pair at a time
        # round r: (1) Q^T/K^T projection for db=r, (2) PV flush + normalize
        # of round r-1, (3) scores + exp/-1/mask chain for r, (4) A-transpose
        # of round r-1 feeding the output projection.
        # Per-head p layout: [q0c0 | q0c1 | q1c0 | q1c1], quadrant j uses
        # keys halo block (qblk+cblk) and mask m0/m1 alternating.
        prev = None  # (db, ptiles{h: pt})

        def pv_flush(pr):
            # merged psum tile: [q0h0 | q0h1 | q1h0 | q1h1], 65 cols each.
            # All full-K accumulate matmuls grouped first, then the K=1 bias
            # matmuls: interleaving row-group-mode switches flushes the PE.
            db, ptl = pr
            pv = ps([128, 4 * (D + 1)])
            for qblk in range(NQB):
                for i, h in enumerate((2 * db, 2 * db + 1)):
                    off = (qblk * 2 + i) * (D + 1)
                    for cblk in range(2):
                        quad = qblk * 2 + cblk
                        nc.tensor.matmul(pv[:, off:off + D + 1],
                                         ptl[h][:, quad * 128:(quad + 1) * 128],
                                         Vaug[:, qblk + cblk, h, :],
                                         start=(qblk == 0 and i == 0
                                                and cblk == 0), stop=False)
            for qblk in range(NQB):
                for i, h in enumerate((2 * db, 2 * db + 1)):
                    off = (qblk * 2 + i) * (D + 1)
                    nc.tensor.matmul(pv[:, off:off + D + 1], ones_sb[0:1, :],
                                     biascat[0:1, h, :], start=False,
                                     stop=(qblk == 1 and i == 1))
            zinv = zpool.tile([128, 4], F32, tag="z", name="zinv")
            zsrc = pv[:].rearrange("p (a z) -> p a z", z=D + 1)[:, :, D]
            nc.vector.reciprocal(zinv[:], zsrc)
            for qblk in range(NQB):
                for i, h in enumerate((2 * db, 2 * db + 1)):
                    j = qblk * 2 + i
                    off = j * (D + 1)
                    if i == 0:
                        nc.scalar.activation(Asc[:, qblk, h * D:(h + 1) * D],
                                             pv[:, off:off + D],
                                             mybir.ActivationFunctionType.Copy,
                                             scale=zinv[:, j:j + 1])
                    else:
                        nc.vector.tensor_scalar_mul(
                            Asc[:, qblk, h * D:(h + 1) * D],
                            pv[:, off:off + D], zinv[:, j:j + 1])

        def a_transpose(db):
            tp = ps([128, 256], BF16)
            for qblk in range(NQB):
                nc.tensor.transpose(tp[:, qblk * 128:(qblk + 1) * 128],
                                    Asc[:, qblk, db * 128:(db + 1) * 128],
                                    identity[:])
            nc.vector.tensor_copy(AT[:, db, :], tp[:])

        prev = None
        for r in range(8 + 1):
            if r < 8:
                db = r
                if r > 0:  # proj(0) was interleaved into the V phase
                    proj(db)
                if prev is not None:
                    pv_flush(prev)
                ptl = sblock(db)
                # lag-2 A-transpose: Asc(r-2) was normalized a full round
                # ago, so the transpose never waits on the recip/scale chain
                if r >= 2:
                    a_transpose(r - 2)
                prev = (db, ptl)
            else:
                # epilogue: fill the PE while round 7's softmax chain and
                # pv/normalize complete.  qblk-major so qblk0's copy-out +
                # DMA overlap qblk1's projection matmuls.
                a_transpose(6)
                opt = {}

                def oproj(qblk, at, start):
                    opp = opt[qblk]
                    for hf in range(2):
                        nc.tensor.matmul(
                            opp[hf][:],
                            AT[:, at, qblk * 128:(qblk + 1) * 128],
                            wo_t[at][:, hf * 512:(hf + 1) * 512],
                            start=start, stop=False)

                def ofinish(qblk):
                    # per-hf pipeline: bias-stop, cast, then two quarter
                    # DMAs per half so four transfers fly concurrently
                    opp = opt[qblk]
                    r0 = qblk * 128
                    ob = obpool.tile([128, E], BF16, tag="ob")
                    nc.tensor.matmul(opp[0][:], ones_sb[0:1, :],
                                     bo_sb[0:1, 0:512], start=False, stop=True)
                    nc.vector.tensor_copy(ob[:, 0:512], opp[0][:])
                    nc.sync.dma_start(out_d[r0:r0 + 128, 0:256], ob[:, 0:256])
                    nc.scalar.dma_start(out_d[r0:r0 + 128, 256:512],
                                        ob[:, 256:512])
                    nc.tensor.matmul(opp[1][:], ones_sb[0:1, :],
                                     bo_sb[0:1, 512:1024], start=False,
                                     stop=True)
                    nc.scalar.copy(ob[:, 512:1024], opp[1][:])
                    nc.sync.dma_start(out_d[r0:r0 + 128, 512:768],
                                      ob[:, 512:768])
                    nc.scalar.dma_start(out_d[r0:r0 + 128, 768:1024],
                                        ob[:, 768:1024])

                opt[0] = [ps([128, 512]) for _ in range(2)]
                for at in range(6):
                    oproj(0, at, at == 0)
                pv_flush(prev)
                oproj(0, 6, False)
                a_transpose(7)
                oproj(0, 7, False)
                ofinish(0)
                opt[1] = [ps([128, 512]) for _ in range(2)]
                for at in range(8):
                    oproj(1, at, at == 0)
                ofinish(1)

    nc.compile()
    return nc


_NC = None


def get_nc():
    global _NC
    if _NC is None:
        _NC = build_graph()
    return _NC


def make_in_maps(x, Wq, bq, Wk, bk, Wv, bv, Wo, bo):
    f = lambda a: np.ascontiguousarray(np.asarray(a, dtype=np.float32))
    bf = lambda a: np.ascontiguousarray(
        np.asarray(a, dtype=np.float32).astype(NPBF16))
    x2 = f(x).reshape(N, E)
    ci = np.arange(128, dtype=np.float32)[:, None]  # key index c (partitions)
    qi = np.arange(128, dtype=np.float32)[None, :]  # query index q (free)
    m0 = (ci >= qi).astype(np.float32)
    m1 = (ci <= qi).astype(np.float32)
    mask4 = np.concatenate([m0, m1, m0, m1], axis=1)
    # host folds: sum_all V_j = xsum @ Wv (per-head bias row, with the +N
    # denominator count), and bo' = bv @ Wo + bo.
    sv = (x2.sum(0, dtype=np.float32) @ f(Wv)).reshape(H, D)
    biascat = np.concatenate(
        [sv, np.full((H, 1), float(N), np.float32)], axis=1).reshape(1, -1)
    bo2 = f(bv) @ f(Wo) + f(bo)
    common = {
        "Wq": bf(Wq), "Wk": bf(Wk), "Wv": bf(Wv), "Wo": bf(Wo),
        "bq_r": f(bq).reshape(8, 128).T.copy(),
        "bk_row": bf(bk).reshape(1, H * D),
        "bo_row": bf(bo2).reshape(1, E),
        "biascat_row": bf(biascat),
        "mask4": bf(mask4),
        "ident": np.eye(128, dtype=np.float32).astype(NPBF16),
    }
    in_maps = []
    for c in range(8):
        r0 = c * R
        xh = np.zeros((HALO, E), NPBF16)
        valid = np.zeros((1, HALO), NPBF16)
        lo, hi = r0 - 64, r0 + R + 64
        slo, shi = max(lo, 0), min(hi, N)
        xh[slo - lo: shi - lo] = x2[slo:shi].astype(NPBF16)
        valid[0, slo - lo: shi - lo] = 1.0
        in_maps.append({**common, "xh": xh, "xvalid": valid})
    return in_maps


def kernel(x, Wq, bq, Wk, bk, Wv, bv, Wo, bo, _trace=False, _trace_kwargs=None):
    nc = get_nc()
    in_maps = make_in_maps(x, Wq, bq, Wk, bk, Wv, bv, Wo, bo)
    res = run_bass_kernel_spmd(nc, in_maps, list(range(8)), trace=_trace,
                               **(_trace_kwargs or {}))
    out = np.concatenate([np.asarray(res.results[c]["out"]) for c in range(8)],
                         axis=0)
    kernel.last_result = res
    return out[None].astype(np.float32)
